# revision 15
# baseline (speedup 1.0000x reference)
"""Channel-attention (XCA over 7x7 windows) Trainium2 Bass/Tile kernel, 8 cores.

Sharding: 8 shards = batch(2) x 4 horizontal strips of 56 rows. Each core gets
its strip plus a 1-row halo (zero-padded at image edges), computes
qkv(1x1) -> depthwise 3x3 -> windowed channel attention -> proj(1x1) locally
in bf16 (fp32 accumulation in PSUM), and writes its 56-row fp32 output strip.
Weights are replicated; no collectives.

Device pipeline per core (4 tiles of 14 output rows):
  - x tile loaded with SWDGE cast fp32->bf16, channels on partitions
  - qkv: 1x1 conv as matmuls (contraction over channels)
  - depthwise 3x3: 9 PSUM-accumulated matmuls per block with diagonal weight
    matrices diag(dw[:, tap]) built on device
  - q,k: squared window sums -> 1/max(sqrt, eps) (q also * temperature), applied
    in a window-padded layout; DMA-transposed (X-bar) to [s, channel] per
    window pair (win A at partitions 0-48, win B at 64-112)
  - attention per (window-pair, head): attnT = k^T q matmuls, exp on ACT,
    softmax denominators via ones-matmuls into channel-block layout,
    out^T = v^T exp matmuls with v partition-aligned via shift DMAs
  - out^T DMA-transposed back to channel layout, normalized by 1/denominator
  - proj: 1x1 conv matmuls from the window-padded layout, fp32 out
"""
import os
import numpy as np
import ml_dtypes

import concourse.bass as bass
import concourse.tile as tile
from concourse import mybir
from concourse.masks import make_identity
from concourse.vector_clock import ScopedClock

# ----------------------------------------------------------------------------
# Workaround: this container's walrus rejects >1 sem wait per instruction.
# Split excess waits onto chained drains (tail) / preceding NoOps (general).
# ----------------------------------------------------------------------------
_MAXW = 1


def _patched_drain_and_barrier(self, tick_clock, wait_clock):
    nc = self.nc
    drain_inst = nc.sync.drain()
    wait_clock.add_sem_waits(
        drain_inst.ins, ScopedClock({None: tick_clock.global_clock})
    )
    si = drain_inst.ins.sync_info
    waits = list(si.on_wait) if si is not None and si.on_wait else []
    if len(waits) > _MAXW:
        si.on_wait = waits[:_MAXW]
        drain_inst.ins.sync_info = si
        for i in range(_MAXW, len(waits), _MAXW):
            extra = nc.sync.drain()
            esi = extra.ins.sync_info
            if esi is None:
                esi = si
            esi.on_wait = waits[i : i + _MAXW]
            esi.on_update = []
            extra.ins.sync_info = esi
    nc.all_engine_barrier()
    assert self.sems is not None
    popped = nc._tile_sem_poison_stack.pop()
    assert popped is self._sem_poison
    nc.clear_and_free_semaphores(list(self.sems.allocated().values()))
    nc.all_engine_barrier()


tile.TileContext._drain_and_barrier = _patched_drain_and_barrier


def _split_sync_waits(nc, maxw=_MAXW):
    n_split = 0
    for f in nc.m.functions:
        for blk in f.blocks:
            insts = blk.instructions
            i = 0
            while i < len(insts):
                inst = insts[i]
                si = inst.sync_info
                if si is not None and si.on_wait and len(si.on_wait) > maxw:
                    waits = list(si.on_wait)
                    keep, extra = waits[:maxw], waits[maxw:]
                    for j in range(0, len(extra), maxw):
                        nop = mybir.InstNoOp(
                            name=nc.get_next_instruction_name(),
                            sync_info=mybir.SyncInfo(
                                on_wait=extra[j : j + maxw], on_update=[]
                            ),
                            bass_nofuse=True,
                            engine=inst.engine,
                        )
                        nc.register_instruction(nop)
                        insts.insert(i, nop)
                        i += 1
                        n_split += 1
                    si.on_wait = keep
                    inst.sync_info = si
                i += 1
    return n_split


# ----------------------------------------------------------------------------
# Geometry
# ----------------------------------------------------------------------------
B, C, H, W = 2, 384, 224, 224
HEADS, PS, D = 8, 7, 48
NCORES = 8
NSTRIP = 4
SH = 56              # output rows per core strip
SHH = SH + 2         # strip rows incl halo
NT, TR = 8, 7        # tiles per strip, output rows per tile (one window row)
TQ = TR + 2          # qkv rows per tile (halo)
NB, MB = 3, 9        # channel blocks in / qkv blocks out
NWW = 32             # window columns
WTILE = NWW          # windows per tile
NPAIR = 16           # window pairs per tile
SS = PS * PS         # 49

f32 = mybir.dt.float32
bf16 = mybir.dt.bfloat16
AF = mybir.ActivationFunctionType
ALU = mybir.AluOpType

# v shift-DMA table: (v block, src p0, src p1, head pair, dst base partition)
_V_SHIFTS = [
    (0, 0, 48, 0, 0), (0, 48, 96, 0, 64), (0, 96, 128, 1, 0),
    (1, 0, 16, 1, 32), (1, 16, 64, 1, 64), (1, 64, 112, 2, 0),
    (1, 112, 128, 2, 64),
    (2, 0, 32, 2, 80), (2, 32, 80, 3, 0), (2, 80, 128, 3, 64),
]


def build_nc():
    PHASE = int(os.environ.get("KPHASE", "9"))
    NTRUN = int(os.environ.get("KNT", str(NT)))
    KSWAPS = int(os.environ.get("KSWAPS", "1"))
    KS64 = int(os.environ.get("KS64", "1"))
    KNPAIR = int(os.environ.get("KNPAIR", str(NPAIR)))
    KE2MODE = int(os.environ.get("KE2MODE", "0"))
    nc = bass.Bass("TRN2", target_bir_lowering=False, debug=False,
                   num_devices=NCORES)
    x_in = nc.dram_tensor("x", [C, SHH, W], f32, kind="ExternalInput").ap()
    qkvwT = nc.dram_tensor("qkvwT", [C, 3 * C], bf16, kind="ExternalInput").ap()
    projwT = nc.dram_tensor("projwT", [C, C], bf16, kind="ExternalInput").ap()
    dww = nc.dram_tensor("dww", [3 * C, 9], f32, kind="ExternalInput").ap()
    tvec = nc.dram_tensor("tvec", [C, 1], f32, kind="ExternalInput").ap()
    y_out = nc.dram_tensor("y", [C, SH, W], f32, kind="ExternalOutput").ap()

    with tile.TileContext(nc) as tc:
        with (
            tc.tile_pool(name="const", bufs=1) as cpool,
            tc.tile_pool(name="xp", bufs=2) as xp,
            tc.tile_pool(name="qkvp", bufs=2) as qkvp,
            tc.tile_pool(name="rawp", bufs=2) as rawp,
            tc.tile_pool(name="sqp", bufs=1) as sqp,
            tc.tile_pool(name="redp", bufs=2) as redp,
            tc.tile_pool(name="qpadp", bufs=6) as qpadp,
            tc.tile_pool(name="vpp", bufs=4) as vpp,
            tc.tile_pool(name="qtp", bufs=3) as qtp,
            tc.tile_pool(name="outcp", bufs=2) as outcp,
            tc.tile_pool(name="yp", bufs=2) as yp,
            tc.tile_pool(name="ps", bufs=3, space="PSUM") as psp,
            tc.tile_pool(name="aps", bufs=4, space="PSUM") as apsp,
            tc.tile_pool(name="sps", bufs=1, space="PSUM") as spsp,
        ):
            # ---------------- weights / constants ----------------
            qkvw_sb, projw_sb, diag_sb, temp_vec = [], [], [], []
            for kb in range(NB):
                t = cpool.tile([128, 3 * C], bf16, tag=f"qkvw{kb}")
                nc.sync.dma_start(out=t, in_=qkvwT[kb * 128:(kb + 1) * 128, :])
                qkvw_sb.append(t)
                t = cpool.tile([128, C], bf16, tag=f"projw{kb}")
                nc.sync.dma_start(out=t, in_=projwT[kb * 128:(kb + 1) * 128, :])
                projw_sb.append(t)
                t = cpool.tile([128, 1], f32, tag=f"tv{kb}")
                nc.sync.dma_start(out=t, in_=tvec[kb * 128:(kb + 1) * 128, :])
                temp_vec.append(t)
            ident = cpool.tile([128, 128], bf16, tag="ident")
            make_identity(nc, ident)
            for m in range(MB):
                dwt = cpool.tile([128, 9], f32, tag=f"dw{m}")
                nc.sync.dma_start(out=dwt, in_=dww[m * 128:(m + 1) * 128, :])
                dg = cpool.tile([128, 9, 128], bf16, tag=f"diag{m}")
                for tap in range(9):
                    nc.vector.tensor_scalar_mul(
                        out=dg[:, tap, :], in0=ident, scalar1=dwt[:, tap:tap + 1])
                diag_sb.append(dg)
            ones = cpool.tile([128, 2], bf16, tag="ones")
            nc.vector.memset(ones, 1.0)

            # ---------------- main loop over row tiles ----------------
            for t in range(NTRUN):
                r0 = t * TR  # strip-padded row of tile row 0

                # x load + cast (SWDGE)
                xbf = []
                for kb in range(NB):
                    xt = xp.tile([128, TQ, W], bf16, tag=f"xbf{kb}")
                    nc.gpsimd.dma_start(
                        out=xt,
                        in_=x_in[kb * 128:(kb + 1) * 128, r0:r0 + TQ, :])
                    xbf.append(xt)

                qpad = {}    # (qk, b) -> tile
                vpack = []   # vb -> tile
                qkv_chunks = [(0, 2), (2, 2), (4, 2), (6, 2), (8, 1)]
                dw_chunks = [(0, 2), (2, 2), (4, 2), (6, 1)]
                for m in range(MB):
                    # ---- qkv projection for block m ----
                    qkv_t = qkvp.tile([128, TQ, W + 2], bf16, tag="qkv")
                    nc.vector.memset(qkv_t[:, :, 0:1], 0.0)
                    nc.vector.memset(qkv_t[:, :, W + 1:W + 2], 0.0)
                    for (rlo, nr) in qkv_chunks:
                        ps_t = psp.tile([128, 448], f32, tag="ps")
                        pp = ps_t[:, 0:nr * W]
                        for kb in range(NB):
                            nc.tensor.matmul(
                                pp,
                                lhsT=qkvw_sb[kb][:, m * 128:(m + 1) * 128],
                                rhs=xbf[kb][:, rlo:rlo + nr, :],
                                start=(kb == 0), stop=(kb == NB - 1))
                        nc.scalar.copy(
                            out=qkv_t[:, rlo:rlo + nr, 1:W + 1], in_=pp)

                    # ---- depthwise 3x3 for block m ----
                    if PHASE < 2:
                        continue
                    if m < 6:
                        raw = rawp.tile([128, TR, W], bf16, tag="qkraw")
                    else:
                        raw = rawp.tile([128, WTILE * SS], bf16, tag="vpack",
                                        bufs=4)
                    for (rlo, nr) in dw_chunks:
                        ps_t = psp.tile([128, 448], f32, tag="ps")
                        pp = ps_t[:, 0:nr * W]
                        for tap in range(9):
                            di, dj = tap // 3, tap % 3
                            nc.tensor.matmul(
                                pp,
                                lhsT=diag_sb[m][:, tap, :],
                                rhs=qkv_t[:, rlo + di:rlo + di + nr,
                                          dj:dj + W],
                                start=(tap == 0), stop=(tap == 8))
                        if m < 6:
                            nc.scalar.copy(
                                out=raw[:, rlo:rlo + nr, :], in_=pp)
                        else:
                            # win-packed: dst off = wj*49 + ri*7 + rj
                            dst = bass.AP(
                                tensor=raw.tensor,
                                offset=raw.offset + rlo * PS,
                                ap=[list(raw.ap[0]),
                                    [PS, nr], [SS, NWW], [1, PS]])
                            nc.scalar.copy(
                                out=dst,
                                in_=pp.rearrange("p (r w j) -> p r w j",
                                                 r=nr, w=NWW))

                    if PHASE < 3:
                        continue
                    if m < 6:
                        # ---- q/k: window norms + apply + pad ----
                        qk, b = (0, m) if m < 3 else (1, m - 3)
                        sq = sqp.tile([128, TR, W], bf16, tag="sq")
                        nc.vector.tensor_mul(sq, raw, raw)
                        r1 = redp.tile([128, TR, NWW], f32, tag="r1")
                        nc.vector.tensor_reduce(
                            out=r1,
                            in_=sq.rearrange("p r (w c) -> p r w c", c=PS),
                            axis=mybir.AxisListType.X, op=ALU.add)
                        rn = redp.tile([128, NWW], f32, tag="rn")
                        nc.vector.tensor_reduce(
                            out=rn,
                            in_=r1.rearrange("p ri w -> p w ri"),
                            axis=mybir.AxisListType.X, op=ALU.add)
                        nc.scalar.sqrt(out=rn, in_=rn)
                        nc.vector.tensor_scalar_max(out=rn, in0=rn,
                                                    scalar1=1e-12)
                        nc.vector.reciprocal(out=rn, in_=rn)
                        if qk == 0:
                            nc.vector.tensor_scalar_mul(
                                out=rn, in0=rn, scalar1=temp_vec[b])
                        qp = qpadp.tile([128, NWW, 64], bf16, tag="qpad")
                        nc.vector.tensor_tensor(
                            out=qp[:, :, 0:SS].rearrange(
                                "p w (ri rj) -> p w ri rj", rj=PS),
                            in0=raw.rearrange(
                                "p ri (w rj) -> p w ri rj", rj=PS),
                            in1=rn.unsqueeze(2).unsqueeze(3)
                                .broadcast_to([128, NWW, PS, PS]),
                            op=ALU.mult)
                        qpad[(qk, b)] = qp
                    else:
                        vpack.append(raw)

                # ---- v partition-alignment shift DMAs ----
                if PHASE < 4:
                    continue
                vpair = [vpp.tile([128, WTILE * SS], bf16, tag=f"vpair{hp}",
                                  name=f"vpair{hp}")
                         for hp in range(4)]
                for (vb, p0, p1, hp, db) in _V_SHIFTS:
                    nc.sync.dma_start(
                        out=vpair[hp][db:db + (p1 - p0), :],
                        in_=vpack[vb][p0:p1, :])

                # ---- attention over 16 window pairs ----
                if PHASE < 5:
                    continue
                sums_ps = spsp.tile([128, 6 * NWW], f32, tag="sums")
                outc = {}
                for j in range(KNPAIR):
                    qT_t = qtp.tile([128, 3 * 128], bf16, tag="qT")
                    kT_t = qtp.tile([128, 3 * 128], bf16, tag="kT")
                    for b in range(NB):
                        nc.sync.dma_start_transpose(
                            out=qT_t[:, b * 128:(b + 1) * 128],
                            in_=qpad[(0, b)][:, 2 * j:2 * j + 2, :])
                        nc.sync.dma_start_transpose(
                            out=kT_t[:, b * 128:(b + 1) * 128],
                            in_=qpad[(1, b)][:, 2 * j:2 * j + 2, :])
                    if PHASE < 6:
                        continue
                    attn_ps = apsp.tile([128, 384], f32, tag="aps")
                    for h in range(HEADS):
                        c0 = D * h
                        nc.tensor.matmul(
                            attn_ps[0:48, c0:c0 + D],
                            lhsT=kT_t[0:SS, c0:c0 + D],
                            rhs=qT_t[0:SS, c0:c0 + D],
                            start=True, stop=True)
                        nc.tensor.matmul(
                            attn_ps[64:112, c0:c0 + D],
                            lhsT=kT_t[64:64 + SS, c0:c0 + D],
                            rhs=qT_t[64:64 + SS, c0:c0 + D],
                            start=True, stop=True)
                    expT = qtp.tile([128, 384], bf16, tag="expT", bufs=2)
                    nc.scalar.activation(out=expT, in_=attn_ps, func=AF.Exp)
                    expsw = qtp.tile([128, 384], bf16, tag="expsw", bufs=2)
                    if KSWAPS:
                        nc.sync.dma_start(out=expsw[64:128, :], in_=expT[0:64, :])
                        nc.sync.dma_start(out=expsw[0:64, :], in_=expT[64:128, :])
                    if PHASE < 7:
                        continue
                    for b in range(NB):
                        col = 2 * (b * NWW + 2 * j)
                        nc.tensor.matmul(
                            sums_ps[:, col:col + 2],
                            lhsT=expT[0:48, b * 128:(b + 1) * 128],
                            rhs=ones[0:48, :], start=True, stop=True)
                        if KS64:
                            # window B sums via the swapped copy: B's e-rows
                            # sit at partitions 0-47 of expsw -> base-0 MM
                            nc.tensor.matmul(
                                sums_ps[:, col + 2:col + 4],
                                lhsT=expsw[0:48, b * 128:(b + 1) * 128],
                                rhs=ones[0:48, :], start=True, stop=True)
                    if PHASE < 8:
                        continue
                    # einsum2: only diagonal tile positions (0,0)/(64,64) --
                    # mixed positions hang the PE on this silicon. Window A
                    # results land in outT_A (even heads at partitions 0-48,
                    # odd heads at 64-112), window B in outT_B; partition
                    # parity is unified afterwards via shift DMAs.
                    outT_A = apsp.tile([128, 384], f32, tag="aps")
                    outT_B = apsp.tile([128, 384], f32, tag="aps")
                    w0 = 2 * j
                    for h in range(HEADS):
                        par, hp, c0 = h % 2, h // 2, D * h
                        bs = 64 * par
                        eA = expT if par == 0 else expsw
                        eB = expsw if par == 0 else expT
                        nc.tensor.matmul(
                            outT_A[bs:bs + SS, c0:c0 + D],
                            lhsT=vpair[hp][bs:bs + 48,
                                           w0 * SS:(w0 + 1) * SS],
                            rhs=eA[bs:bs + 48, c0:c0 + D],
                            start=True, stop=True)
                        nc.tensor.matmul(
                            outT_B[bs:bs + SS, c0:c0 + D],
                            lhsT=vpair[hp][bs:bs + 48,
                                           (w0 + 1) * SS:(w0 + 2) * SS],
                            rhs=eB[bs:bs + 48, c0:c0 + D],
                            start=True, stop=True)
                    stageA = qtp.tile([128, 384], bf16, tag="outTs", bufs=2)
                    nc.scalar.copy(out=stageA, in_=outT_A)
                    stageB = qtp.tile([128, 384], bf16, tag="outTs2", bufs=2)
                    nc.scalar.copy(out=stageB, in_=outT_B)
                    unif = qtp.tile([128, 384], bf16, tag="unif", bufs=2)
                    sAv = stageA.rearrange("p (hp par d) -> p hp par d",
                                           hp=4, par=2)
                    sBv = stageB.rearrange("p (hp par d) -> p hp par d",
                                           hp=4, par=2)
                    uv = unif.rearrange("p (hp par d) -> p hp par d",
                                        hp=4, par=2)
                    nc.sync.dma_start(out=uv[0:64, :, 0, :],
                                      in_=sAv[0:64, :, 0, :])
                    nc.sync.dma_start(out=uv[0:64, :, 1, :],
                                      in_=sAv[64:128, :, 1, :])
                    nc.sync.dma_start(out=uv[64:128, :, 0, :],
                                      in_=sBv[0:64, :, 0, :])
                    nc.sync.dma_start(out=uv[64:128, :, 1, :],
                                      in_=sBv[64:128, :, 1, :])
                    for b in range(NB):
                        if j == 0:
                            oc = outcp.tile([128, NWW, 64], bf16,
                                            tag=f"outc{b}", name=f"outc{b}")
                            outc[b] = oc
                        nc.sync.dma_start_transpose(
                            out=outc[b][:, 2 * j:2 * j + 2, :],
                            in_=unif[:, b * 128:(b + 1) * 128])

                # ---- normalize + proj + store ----
                if PHASE < 9:
                    continue
                rden = redp.tile([128, NB, NWW], f32, tag="rden")
                for b in range(NB):
                    nc.vector.reciprocal(
                        out=rden[:, b, :],
                        in_=sums_ps[:, 2 * b * NWW:2 * (b + 1) * NWW]
                            .rearrange("p (w two) -> p w two", two=2)[:, :, 0])
                rdbf = redp.tile([128, NB, NWW], bf16, tag="rdbf")
                nc.vector.tensor_copy(out=rdbf, in_=rden)
                for b in range(NB):
                    nc.vector.tensor_tensor(
                        out=outc[b][:, :, 0:SS], in0=outc[b][:, :, 0:SS],
                        in1=rdbf[:, b, :].unsqueeze(2)
                            .broadcast_to([128, NWW, SS]),
                        op=ALU.mult)
                for mb in range(NB):
                    ysb = yp.tile([128, PS, W], f32, tag="ysb")
                    for n in range(4):
                        ps_t = psp.tile([128, 448], f32, tag="ps")
                        pp = ps_t[:, 0:8 * SS]
                        for kb in range(NB):
                            nc.tensor.matmul(
                                pp,
                                lhsT=projw_sb[kb][:, mb * 128:(mb + 1) * 128],
                                rhs=outc[kb][:, 8 * n:8 * n + 8, 0:SS],
                                start=(kb == 0), stop=(kb == NB - 1))
                        nc.vector.tensor_copy(
                            out=ysb[:, :, 56 * n:56 * (n + 1)].rearrange(
                                "p s (w r) -> p w s r", r=PS),
                            in_=pp.rearrange("p (w s r) -> p w s r",
                                             w=8, s=PS))
                    nc.sync.dma_start(
                        out=y_out[mb * 128:(mb + 1) * 128,
                                  t * TR:(t + 1) * TR, :],
                        in_=ysb)

    _split_sync_waits(nc)
    return nc


# ----------------------------------------------------------------------------
# Host side
# ----------------------------------------------------------------------------
_STATE: dict = {}


def _make_shards(x):
    """x [B, C, H, W] fp32 -> [8, C, 58, 224] strips with halo."""
    xp_ = np.pad(np.asarray(x, dtype=np.float32),
                 ((0, 0), (0, 0), (1, 1), (0, 0)))
    shards = np.empty((NCORES, C, SHH, W), dtype=np.float32)
    for b in range(B):
        for s in range(NSTRIP):
            shards[b * NSTRIP + s] = xp_[b, :, s * SH:s * SH + SHH, :]
    return shards


def _prep_weights(qkv_w, dw_w, temperature, proj_w):
    qkvwT = np.ascontiguousarray(
        np.asarray(qkv_w, np.float32).T).astype(ml_dtypes.bfloat16)
    projwT = np.ascontiguousarray(
        np.asarray(proj_w, np.float32).T).astype(ml_dtypes.bfloat16)
    dww = np.asarray(dw_w, np.float32).reshape(3 * C, 9).copy()
    tvec = np.repeat(np.asarray(temperature, np.float32).reshape(HEADS),
                     D).reshape(C, 1).copy()
    return qkvwT, projwT, dww, tvec


def _compiled():
    if _STATE:
        return _STATE
    import jax
    import jax.numpy as jnp
    from jax.sharding import Mesh, PartitionSpec, NamedSharding
    from jax.experimental.shard_map import shard_map
    import concourse.bass2jax as b2j

    b2j.install_neuronx_cc_hook()
    nc = build_nc()

    partition_name = (nc.partition_id_tensor.name
                      if nc.partition_id_tensor else None)
    in_names, out_names, out_avals, zero_shapes = [], [], [], []
    for alloc in nc.m.functions[0].allocations:
        if not isinstance(alloc, mybir.MemoryLocationSet):
            continue
        name = alloc.memorylocations[0].name
        if alloc.kind == "ExternalInput":
            if name != partition_name:
                in_names.append(name)
        elif alloc.kind == "ExternalOutput":
            shape = tuple(alloc.tensor_shape)
            dtype = mybir.dt.np(alloc.dtype)
            out_names.append(name)
            out_avals.append(jax.core.ShapedArray(shape, dtype))
            zero_shapes.append((shape, dtype))
    n_params = len(in_names)
    n_outs = len(out_names)
    all_names = list(in_names) + list(out_names)
    if partition_name is not None:
        all_names.append(partition_name)
    donate = tuple(range(n_params, n_params + n_outs))

    def _body(*args):
        operands = list(args)
        if partition_name is not None:
            operands.append(b2j.partition_id_tensor())
        outs = b2j._bass_exec_p.bind(
            *operands,
            out_avals=tuple(out_avals),
            in_names=tuple(all_names),
            out_names=tuple(out_names),
            lowering_input_output_aliases=(),
            sim_require_finite=False,
            sim_require_nnan=False,
            nc=nc,
        )
        return tuple(outs)

    devices = jax.devices()[:NCORES]
    mesh = Mesh(np.asarray(devices), ("core",))
    in_specs = (PartitionSpec("core"),) * (n_params + n_outs)
    out_specs = (PartitionSpec("core"),) * n_outs
    fn = jax.jit(
        shard_map(_body, mesh=mesh, in_specs=in_specs, out_specs=out_specs,
                  check_rep=False),
        donate_argnums=donate, keep_unused=True)

    sharding = NamedSharding(mesh, PartitionSpec("core"))

    def make_zeros():
        outs = []
        for shape, dtype in zero_shapes:
            gshape = (NCORES * shape[0],) + tuple(shape[1:])
            outs.append(jax.jit(
                lambda s=gshape, d=dtype: jnp.zeros(s, d),
                out_shardings=sharding)())
        return outs

    _STATE.update(nc=nc, fn=fn, in_names=in_names, out_names=out_names,
                  out_avals=out_avals, make_zeros=make_zeros, mesh=mesh,
                  sharding=sharding, jax=jax)
    return _STATE


def _device_inputs(inputs):
    """inputs dict (full problem inputs) -> list of concatenated global arrays
    in in_names order, placed on the device mesh."""
    import jax
    st = _compiled()
    shards = _make_shards(inputs["x"])
    qkvwT, projwT, dww, tvec = _prep_weights(
        inputs["qkv_w"], inputs["dw_w"], inputs["temperature"],
        inputs["proj_w"])
    per_core = {
        "x": [shards[i] for i in range(NCORES)],
        "qkvwT": [qkvwT] * NCORES,
        "projwT": [projwT] * NCORES,
        "dww": [dww] * NCORES,
        "tvec": [tvec] * NCORES,
    }
    arrs = []
    for name in st["in_names"]:
        arrs.append(np.concatenate(per_core[name], axis=0))
    return [jax.device_put(a, st["sharding"]) for a in arrs]


def _run_device(dev_inputs):
    st = _compiled()
    zeros = st["make_zeros"]()
    outs = st["fn"](*dev_inputs, *zeros)
    return outs


def kernel(x, qkv_w, dw_w, temperature, proj_w):
    st = _compiled()
    dev_inputs = _device_inputs(dict(x=x, qkv_w=qkv_w, dw_w=dw_w,
                                     temperature=temperature, proj_w=proj_w))
    outs = _run_device(dev_inputs)
    y_g = np.asarray(outs[st["out_names"].index("y")])  # [8*C, 56, 224]
    y_g = y_g.reshape(NCORES, C, SH, W)
    out = np.empty((B, C, H, W), dtype=np.float32)
    for b in range(B):
        for s in range(NSTRIP):
            out[b, :, s * SH:(s + 1) * SH, :] = y_g[b * NSTRIP + s]
    return out
